# revision 15
# baseline (speedup 1.0000x reference)
"""Trainium2 Bass kernel: quantized BasicBlock (quant-conv3x3 -> bn -> relu ->
quant-conv3x3 -> bn -> +residual -> relu).

Sharding: data-parallel over the batch dim of x across 8 NeuronCores (8 images
per core).  Weight quantization (centroid/deviation pipeline) is replicated on
every core, computed on-device.

Math notes:
  - jnp.round (round-half-even) is implemented with the fp32 magic-number
    trick: rne(v) = (v + 1.5*2^23) - 1.5*2^23 for |v| < 2^22.
  - Quantized weights are integer "levels" dev+cent = k/8 with |k| < 2048,
    exactly representable in fp16.  The global scale `step` is folded into the
    BN scale vector, so matmuls run in fp16 (4x faster than fp32 on the PE)
    with fp32 PSUM accumulation and no weight-precision loss.
"""

import sys

for _p in ("/opt/trn_rl_repo",):
    if _p not in sys.path:
        sys.path.insert(0, _p)

from contextlib import ExitStack

import numpy as np

import concourse.bass as bass
import concourse.tile as tile
from concourse import bacc, bass_isa, mybir
from concourse.bass_utils import run_bass_kernel_spmd
from concourse.masks import make_identity

P = 128
B, C, H, W = 64, 256, 28, 28
NCORES = 8
BPC = B // NCORES          # images per core
CK = C // P                # channel chunks (2)
TAPS = 9
HP, WP = H + 2, W + 2      # zero-padded spatial 30x30
NR = H // 2                # rows per psum chunk (14)
NN = NR * W                # matmul free dim (392)
F32 = mybir.dt.float32
F16 = mybir.dt.float16

MAGIC = 12582912.0         # 1.5 * 2**23  (fp32 RNE round-to-int trick)
HALF_LVLS = 127.0
LV = 8.0                   # 2**(NUM_BITS-1)
CSTEP = HALF_LVLS / LV     # 15.875
DEVW = 0.5 * HALF_LVLS     # 63.5
BN_EPS = 1e-5

AF = mybir.ActivationFunctionType
OP = mybir.AluOpType
AX = mybir.AxisListType


def _emit(nc, tc, ctx, td):
    """Emit the whole per-core program.  td: dict of DRAM tensor handles."""
    const = ctx.enter_context(tc.tile_pool(name="const", bufs=1))
    bnp = ctx.enter_context(tc.tile_pool(name="bnp", bufs=2))
    wbig = ctx.enter_context(tc.tile_pool(name="wbig", bufs=1))
    whalf = ctx.enter_context(tc.tile_pool(name="whalf", bufs=3))
    wqp = ctx.enter_context(tc.tile_pool(name="wqp", bufs=1))
    wtp = ctx.enter_context(tc.tile_pool(name="wtp", bufs=1))
    tpp = ctx.enter_context(tc.tile_pool(name="tpp", bufs=2, space="PSUM"))
    psp = ctx.enter_context(tc.tile_pool(name="psp", bufs=6, space="PSUM"))
    pxf = ctx.enter_context(tc.tile_pool(name="pxf", bufs=6))
    pxp = ctx.enter_context(tc.tile_pool(name="pxp", bufs=5))
    phh = ctx.enter_context(tc.tile_pool(name="phh", bufs=5))
    pyy = ctx.enter_context(tc.tile_pool(name="pyy", bufs=3))
    pep = ctx.enter_context(tc.tile_pool(name="pep", bufs=3))

    ident16 = const.tile([P, P], F16, name="ident16", tag="ident16")
    make_identity(nc, ident16)
    ident32 = const.tile([P, P], F32, name="ident32", tag="ident32")
    make_identity(nc, ident32)
    ones32 = const.tile([1, P], F32, name="ones32", tag="ones32")
    nc.gpsimd.memset(ones32[:], 1.0)
    magicv = const.tile([P, 1], F32, name="magicv", tag="magicv")
    nc.gpsimd.memset(magicv[:], MAGIC)
    # warm the ScalarE activation tables during the initial DMA wait so the
    # one-time ACT_TABLE_LOADs don't block the quant chain later
    scr = const.tile([P, 1], F32, name="scr", tag="scr")
    nc.scalar.activation(scr[:], magicv[:], AF.Sqrt)
    nc.scalar.activation(scr[:], magicv[:], AF.Relu)

    wT = {}      # wT[j][k] : [P(ci), CK(m), TAPS, P(co)] fp16
    inv_s = {}   # BN scale with quant step folded in: [P, CK]
    bvec = {}    # BN bias: [P, CK]
    _w32 = {}
    _wq = {}
    _istep = {}
    _inv = {}

    # ---------------- image loads ------------------------------------------
    x_view = td["x"].ap().rearrange("b (c p) h w -> b p c h w", p=P)
    y_view = td["y"].ap().rearrange("b (c p) h w -> b p c h w", p=P)
    xf_t = [None] * BPC
    xp_t = [None] * BPC
    h_t = [None] * BPC

    def load_x(i):
        xf = pxf.tile([P, CK, H, W], F32, name=f"xf{i}", tag="xf")
        nc.sync.dma_start(xf[:], x_view[i])
        xp = pxp.tile([P, CK, HP, WP], F16, name=f"xp{i}", tag="xp")
        nc.gpsimd.memset(xp[:], 0.0)
        nc.scalar.copy(xp[:, :, 1 : 1 + H, 1 : 1 + W], xf[:])
        xf_t[i], xp_t[i] = xf, xp

    # ---------------- per-weight quantization ------------------------------
    _step = {}

    def quant_dma(j):
        """Issue weight DMAs (sync engine only — no compute-engine stalls)."""
        w32 = wbig.tile([P, CK, C, TAPS], F32, name=f"w32_{j}", tag="wbig")
        wsrc = td[f"w{j}"].ap().rearrange("(c p) ci kh kw -> p c ci (kh kw)", p=P)
        for c in range(CK):
            for k in range(CK):
                ks = slice(k * P, (k + 1) * P)
                nc.sync.dma_start(w32[:, c, ks, :], wsrc[:, c, ks, :])
        _w32[j] = w32

    def quant_absmax(j):
        """Global absmax -> step/istep (reduces on DVE, combine on PE)."""
        w32 = _w32[j]
        pmq = []
        for c in range(CK):
            for k in range(CK):
                ks = slice(k * P, (k + 1) * P)
                ph = bnp.tile([P, 1], F32, name=f"pmq{j}_{c}_{k}", tag="pmq")
                nc.vector.tensor_reduce(
                    ph[:], w32[:, c, ks, :], axis=AX.XY, op=OP.max,
                    apply_absolute_value=True,
                )
                pmq.append(ph)
        pa = bnp.tile([P, 1], F32, name=f"pa{j}", tag="pa")
        nc.vector.tensor_max(pa[:], pmq[0][:], pmq[1][:])
        pb = bnp.tile([P, 1], F32, name=f"pb{j}", tag="pb")
        nc.vector.tensor_max(pb[:], pmq[2][:], pmq[3][:])
        pm = bnp.tile([P, 1], F32, name=f"pm{j}", tag="pm")
        nc.vector.tensor_max(pm[:], pa[:], pb[:])
        # cross-partition max via PE: transpose [128,1]->[1,128], reduce,
        # then broadcast back with a K=1 ones matmul (gpsimd ucode is ~10us)
        pmt = tpp.tile([1, P], F32, name=f"pmt{j}", tag="tp")
        nc.tensor.transpose(pmt[:], pm[:], ident32[:])
        sm = bnp.tile([1, 1], F32, name=f"sm{j}", tag="sm")
        nc.vector.tensor_reduce(sm[:], pmt[:], axis=AX.X, op=OP.max)
        pmb = tpp.tile([P, 1], F32, name=f"pmb{j}", tag="tp")
        nc.tensor.matmul(pmb[:], ones32[:], sm[:])
        pmax = bnp.tile([P, 1], F32, name=f"pmax{j}", tag="pmax")
        nc.vector.tensor_copy(pmax[:], pmb[:])
        step = const.tile([P, 1], F32, name=f"step{j}", tag=f"step{j}")
        nc.vector.tensor_scalar_mul(step[:], pmax[:], 1.0 / HALF_LVLS)
        _step[j] = step
        rmax = bnp.tile([P, 1], F32, name=f"rmax{j}", tag="rmax")
        nc.vector.reciprocal(rmax[:], pmax[:])
        istep = const.tile([P, 1], F32, name=f"istep{j}", tag=f"istep{j}")
        nc.vector.tensor_scalar_mul(istep[:], rmax[:], HALF_LVLS)
        _istep[j] = istep
        # fold step into BN scale: inv_s = inv * step
        ivs = const.tile([P, CK], F32, name=f"ivs{j}", tag=f"ivs{j}")
        nc.vector.tensor_scalar_mul(ivs[:], _inv[j][:], _step[j][:, 0:1])
        inv_s[j] = ivs

        wq = wqp.tile([P, CK, C, TAPS], F16, name=f"wq{j}", tag=f"wq{j}")
        _wq[j] = wq
        wT[j] = []
        for k in range(CK):
            wt = wtp.tile([P, CK, TAPS, P], F16, name=f"wT{j}_{k}", tag=f"wT{j}_{k}")
            wT[j].append(wt)

    def bn_prep(j):
        """BN vector prep.  Contiguous [1,256] row loads (one descriptor each
        — the [128,2] gather form is 256 tiny descriptors, ~10us), math on one
        partition, then redistribute to [128,2] via K=1 PE matmuls."""
        gv = bnp.tile([1, C], F32, name=f"gv{j}", tag=f"gv{j}")
        bev = bnp.tile([1, C], F32, name=f"bev{j}", tag=f"bev{j}")
        muv = bnp.tile([1, C], F32, name=f"muv{j}", tag=f"muv{j}")
        vav = bnp.tile([1, C], F32, name=f"vav{j}", tag=f"vav{j}")
        nc.sync.dma_start(gv[:], td[f"gamma{j}"].ap().unsqueeze(0))
        nc.sync.dma_start(bev[:], td[f"beta{j}"].ap().unsqueeze(0))
        nc.sync.dma_start(muv[:], td[f"mean{j}"].ap().unsqueeze(0))
        nc.sync.dma_start(vav[:], td[f"var{j}"].ap().unsqueeze(0))

        tv = bnp.tile([1, C], F32, name=f"tv{j}", tag="btmp")
        nc.vector.tensor_scalar_add(tv[:], vav[:], BN_EPS)
        rv = bnp.tile([1, C], F32, name=f"rv{j}", tag="btmp")
        nc.vector.reciprocal(rv[:], tv[:])
        sv = bnp.tile([1, C], F32, name=f"sv{j}", tag="btmp")
        nc.scalar.activation(sv[:], rv[:], AF.Sqrt)           # rsqrt(var+eps)
        invr = bnp.tile([1, C], F32, name=f"invr{j}", tag=f"invr{j}")
        nc.vector.tensor_mul(invr[:], sv[:], gv[:])           # gamma * rsqrt
        mi = bnp.tile([1, C], F32, name=f"mi{j}", tag="btmp")
        nc.vector.tensor_mul(mi[:], muv[:], invr[:])
        bvr = bnp.tile([1, C], F32, name=f"bvr{j}", tag=f"bvr{j}")
        nc.vector.tensor_sub(bvr[:], bev[:], mi[:])           # beta - mean*inv

        # redistribute rows -> per-partition [P, CK] with K=1 matmuls
        psB = tpp.tile([P, 2 * CK], F32, name=f"psB{j}", tag="tp")
        for c in range(CK):
            nc.tensor.matmul(
                psB[:, c : c + 1], invr[0:1, c * P : (c + 1) * P], ones32[0:1, 0:1]
            )
            nc.tensor.matmul(
                psB[:, CK + c : CK + c + 1],
                bvr[0:1, c * P : (c + 1) * P],
                ones32[0:1, 0:1],
            )
        inv = const.tile([P, CK], F32, name=f"inv{j}", tag=f"inv{j}")
        nc.vector.tensor_copy(inv[:], psB[:, 0:CK])
        bv = const.tile([P, CK], F32, name=f"bv{j}", tag=f"bv{j}")
        nc.vector.tensor_copy(bv[:], psB[:, CK : 2 * CK])
        bvec[j] = bv
        _inv[j] = inv

    def quant_chain(j):
        """Quantization pipeline at (co-chunk, ci-half) granularity so the
        first transposes (and conv matmuls) start as early as possible."""
        w32, wq, istep = _w32[j], _wq[j], _istep[j]
        for c in range(CK):      # co-chunk == psum m chunk
            for k in range(CK):  # ci half (128 input channels)
                ks = slice(k * P, (k + 1) * P)
                src = w32[:, c, ks, :]
                # wl = rne(w * istep); the +-127 clip is redundant: |w*istep|
                # <= 127*(1+2^-23) by construction, and rne of that is 127.
                # Same for the centroid's +-8 clip (|gm|/9/cstep <= 8).
                wlr = whalf.tile([P, P, TAPS], F32, name=f"wlr{j}_{c}_{k}", tag="wh")
                nc.scalar.activation(
                    wlr[:], src, AF.Identity, bias=magicv[:, 0:1], scale=istep[:, 0:1]
                )
                wl3 = whalf.tile([P, P, TAPS], F32, name=f"wl3{j}_{c}_{k}", tag="wh")
                nc.vector.tensor_scalar_sub(wl3[:], wlr[:], MAGIC)

                # per-grain (co, ci) mean over the 9 taps -> centroid levels
                gm = bnp.tile([P, P], F32, name=f"gm{j}_{c}_{k}", tag="gm")
                nc.vector.tensor_reduce(gm[:], wl3[:], axis=AX.X, op=OP.add)
                c1 = bnp.tile([P, P], F32, name=f"c1{j}_{c}_{k}", tag="c1")
                nc.vector.tensor_scalar(
                    c1[:], gm[:], 1.0 / (TAPS * CSTEP), MAGIC, OP.mult, OP.add
                )
                cent = bnp.tile([P, P], F32, name=f"cent{j}_{c}_{k}", tag="cent")
                nc.vector.tensor_scalar(
                    cent[:], c1[:], MAGIC, CSTEP, OP.subtract, OP.mult
                )
                centb = cent.unsqueeze(2).broadcast_to((P, P, TAPS))

                # dev = rne(clip(wl - cent, -63.5, 63.5)); wq = dev + cent
                dv = whalf.tile([P, P, TAPS], F32, name=f"dv{j}_{c}_{k}", tag="wh")
                nc.vector.tensor_sub(dv[:], wl3[:], centb)
                dv2 = whalf.tile([P, P, TAPS], F32, name=f"dv2{j}_{c}_{k}", tag="wh")
                nc.vector.tensor_scalar(dv2[:], dv[:], DEVW, -DEVW, OP.min, OP.max)
                dv3 = whalf.tile([P, P, TAPS], F32, name=f"dv3{j}_{c}_{k}", tag="wh")
                nc.vector.tensor_scalar(
                    dv3[:], dv2[:], MAGIC, MAGIC, OP.add, OP.subtract
                )
                nc.vector.tensor_add(wq[:, c, ks, :], dv3[:], centb)

                # PE-transpose the 9 taps of this (m=c, k): [co,ci] -> [ci,co]
                m = c
                for t0 in (0, 4, 8):
                    nb = min(4, TAPS - t0)
                    pst = tpp.tile(
                        [P, nb, P], F16, name=f"pst{j}_{m}_{k}_{t0}", tag="tp"
                    )
                    for dt in range(nb):
                        nc.tensor.transpose(
                            pst[:, dt, :],
                            wq[:, m, k * P : (k + 1) * P, t0 + dt],
                            ident16[:],
                        )
                    nc.scalar.copy(wT[j][k][:, m, t0 : t0 + nb, :], pst[:])

    # ---------------- convolutions -----------------------------------------
    def conv_mms(ps, src16, wTj, m, r0):
        idx = 0
        for k in range(CK):
            for dh in range(3):
                for dw in range(3):
                    t = dh * 3 + dw
                    nc.tensor.matmul(
                        ps[:],
                        wTj[k][:, m, t, :],
                        src16[:, k, r0 + dh : r0 + dh + NR, dw : dw + W],
                        start=(idx == 0),
                        stop=(idx == 2 * TAPS - 1),
                    )
                    idx += 1

    def conv1(i):
        hh = phh.tile([P, CK, HP, WP], F16, name=f"h{i}", tag="h")
        nc.gpsimd.memset(hh[:], 0.0)
        h_t[i] = hh
        for m in range(CK):
            for r in range(2):
                r0 = r * NR
                ps = psp.tile([P, NN], F32, name=f"ps1_{i}_{m}_{r}", tag="ps")
                conv_mms(ps, xp_t[i], wT[1], m, r0)
                nc.scalar.activation(
                    hh[:, m, 1 + r0 : 1 + r0 + NR, 1 : 1 + W],
                    ps.rearrange("p (r w) -> p r w", w=W),
                    AF.Relu,
                    bias=bvec[1][:, m : m + 1],
                    scale=inv_s[1][:, m : m + 1],
                )

    def conv2(i):
        for m in range(CK):
            yf = pyy.tile([P, H, W], F32, name=f"y{i}_{m}", tag="y")
            for r in range(2):
                r0 = r * NR
                ps = psp.tile([P, NN], F32, name=f"ps2_{i}_{m}_{r}", tag="ps")
                conv_mms(ps, h_t[i], wT[2], m, r0)
                t2 = pep.tile([P, NN], F32, name=f"t2_{i}_{m}_{r}", tag="t2")
                nc.scalar.activation(
                    t2[:],
                    ps[:],
                    AF.Identity,
                    bias=bvec[2][:, m : m + 1],
                    scale=inv_s[2][:, m : m + 1],
                )
                u = pep.tile([P, NN], F32, name=f"u_{i}_{m}_{r}", tag="u")
                xflat = xf_t[i][:, m, r0 : r0 + NR, :].rearrange("p r w -> p (r w)")
                nc.vector.tensor_add(u[:], t2[:], xflat)
                nc.scalar.activation(
                    yf[:, r0 : r0 + NR, :],
                    u.rearrange("p (r w) -> p r w", w=W),
                    AF.Relu,
                )
            nc.gpsimd.dma_start(y_view[i][:, m], yf[:])

    def pe_warmup(n):
        """Junk matmuls (ident16 x broadcast-ident16) to hold the PE HAM at
        K=8/8 through the head's DMA wait, so real matmuls start warm."""
        rhsb = ident16.unsqueeze(1).broadcast_to((P, 4, P))
        for i in range(n):
            scr_ps = tpp.tile([P, 4 * P], F32, name=f"warm{_wuid[0]}", tag="tp")
            _wuid[0] += 1
            nc.tensor.matmul(scr_ps[:], ident16[:], rhsb)

    _wuid = [0]

    # ---------------- emission order (engine priority) ---------------------
    quant_dma(1)
    bn_prep(1)
    bn_prep(2)
    load_x(0)
    load_x(1)
    quant_dma(2)
    for i in range(2, BPC):
        load_x(i)
    pe_warmup(26)
    quant_absmax(1)
    pe_warmup(10)
    quant_chain(1)
    conv1(0)
    conv1(1)
    quant_absmax(2)
    quant_chain(2)
    conv1(2)
    conv1(3)
    for i in range(BPC):
        if i + 4 < BPC:
            conv1(i + 4)
        conv2(i)


def build_bass():
    nc = bacc.Bacc(
        "TRN2", target_bir_lowering=False, debug=False, num_devices=NCORES
    )
    td = {}
    td["x"] = nc.dram_tensor("x", (BPC, C, H, W), F32, kind="ExternalInput")
    for j in (1, 2):
        td[f"w{j}"] = nc.dram_tensor(f"w{j}", (C, C, 3, 3), F32, kind="ExternalInput")
        for v in ("gamma", "beta", "mean", "var"):
            td[f"{v}{j}"] = nc.dram_tensor(f"{v}{j}", (C,), F32, kind="ExternalInput")
    td["y"] = nc.dram_tensor("y", (BPC, C, H, W), F32, kind="ExternalOutput")

    with tile.TileContext(nc) as tc:
        with ExitStack() as ctx:
            _emit(nc, tc, ctx, td)
    nc.compile()
    return nc


_NC = None


def _get_nc():
    global _NC
    if _NC is None:
        _NC = build_bass()
    return _NC


def make_in_maps(x, w1, gamma1, beta1, mean1, var1, w2, gamma2, beta2, mean2, var2):
    rep = {
        "w1": w1, "gamma1": gamma1, "beta1": beta1, "mean1": mean1, "var1": var1,
        "w2": w2, "gamma2": gamma2, "beta2": beta2, "mean2": mean2, "var2": var2,
    }
    rep = {k: np.ascontiguousarray(np.asarray(v), dtype=np.float32) for k, v in rep.items()}
    in_maps = []
    for c in range(NCORES):
        m = {"x": np.ascontiguousarray(np.asarray(x)[c * BPC : (c + 1) * BPC], dtype=np.float32)}
        m.update(rep)
        in_maps.append(m)
    return in_maps


def kernel(x, w1, gamma1, beta1, mean1, var1,
           w2, gamma2, beta2, mean2, var2, codebook=None, **_unused):
    nc = _get_nc()
    in_maps = make_in_maps(x, w1, gamma1, beta1, mean1, var1,
                           w2, gamma2, beta2, mean2, var2)
    res = run_bass_kernel_spmd(nc, in_maps, core_ids=list(range(NCORES)))
    return np.concatenate([r["y"] for r in res.results], axis=0)


# revision 16
# speedup vs baseline: 1.0657x; 1.0657x over previous
"""Trainium2 Bass kernel: quantized BasicBlock (quant-conv3x3 -> bn -> relu ->
quant-conv3x3 -> bn -> +residual -> relu).

Sharding: data-parallel over the batch dim of x across 8 NeuronCores (8 images
per core).  Weight quantization (centroid/deviation pipeline) is replicated on
every core, computed on-device.

Math notes:
  - jnp.round (round-half-even) is implemented with the fp32 magic-number
    trick: rne(v) = (v + 1.5*2^23) - 1.5*2^23 for |v| < 2^22.
  - Quantized weights are integer "levels" dev+cent = k/8 with |k| < 2048,
    exactly representable in fp16.  The global scale `step` is folded into the
    BN scale vector, so matmuls run in fp16 (4x faster than fp32 on the PE)
    with fp32 PSUM accumulation and no weight-precision loss.
"""

import sys

for _p in ("/opt/trn_rl_repo",):
    if _p not in sys.path:
        sys.path.insert(0, _p)

from contextlib import ExitStack

import numpy as np

import concourse.bass as bass
import concourse.tile as tile
from concourse import bacc, bass_isa, mybir
from concourse.bass_utils import run_bass_kernel_spmd
from concourse.masks import make_identity

P = 128
B, C, H, W = 64, 256, 28, 28
NCORES = 8
BPC = B // NCORES          # images per core
CK = C // P                # channel chunks (2)
TAPS = 9
HP, WP = H + 2, W + 2      # zero-padded spatial 30x30
NR = H // 2                # rows per psum chunk (14)
NN = NR * W                # matmul free dim (392)
F32 = mybir.dt.float32
F16 = mybir.dt.float16

MAGIC = 12582912.0         # 1.5 * 2**23  (fp32 RNE round-to-int trick)
HALF_LVLS = 127.0
LV = 8.0                   # 2**(NUM_BITS-1)
CSTEP = HALF_LVLS / LV     # 15.875
DEVW = 0.5 * HALF_LVLS     # 63.5
BN_EPS = 1e-5

AF = mybir.ActivationFunctionType
OP = mybir.AluOpType
AX = mybir.AxisListType


def _emit(nc, tc, ctx, td):
    """Emit the whole per-core program.  td: dict of DRAM tensor handles."""
    const = ctx.enter_context(tc.tile_pool(name="const", bufs=1))
    bnp = ctx.enter_context(tc.tile_pool(name="bnp", bufs=2))
    wbig = ctx.enter_context(tc.tile_pool(name="wbig", bufs=1))
    whalf = ctx.enter_context(tc.tile_pool(name="whalf", bufs=3))
    wqp = ctx.enter_context(tc.tile_pool(name="wqp", bufs=1))
    wtp = ctx.enter_context(tc.tile_pool(name="wtp", bufs=1))
    tpp = ctx.enter_context(tc.tile_pool(name="tpp", bufs=2, space="PSUM"))
    psp = ctx.enter_context(tc.tile_pool(name="psp", bufs=6, space="PSUM"))
    pxf = ctx.enter_context(tc.tile_pool(name="pxf", bufs=6))
    pxp = ctx.enter_context(tc.tile_pool(name="pxp", bufs=5))
    phh = ctx.enter_context(tc.tile_pool(name="phh", bufs=5))
    pyy = ctx.enter_context(tc.tile_pool(name="pyy", bufs=3))
    pep = ctx.enter_context(tc.tile_pool(name="pep", bufs=3))

    ident16 = const.tile([P, P], F16, name="ident16", tag="ident16")
    make_identity(nc, ident16)
    ident32 = const.tile([P, P], F32, name="ident32", tag="ident32")
    make_identity(nc, ident32)
    ones32 = const.tile([1, P], F32, name="ones32", tag="ones32")
    nc.gpsimd.memset(ones32[:], 1.0)
    magicv = const.tile([P, 1], F32, name="magicv", tag="magicv")
    nc.gpsimd.memset(magicv[:], MAGIC)
    # warm the ScalarE activation tables during the initial DMA wait so the
    # one-time ACT_TABLE_LOADs don't block the quant chain later
    scr = const.tile([P, 1], F32, name="scr", tag="scr")
    nc.scalar.activation(scr[:], magicv[:], AF.Sqrt)
    nc.scalar.activation(scr[:], magicv[:], AF.Relu)

    wT = {}      # wT[j][k] : [P(ci), CK(m), TAPS, P(co)] fp16
    inv_s = {}   # BN scale with quant step folded in: [P, CK]
    bvec = {}    # BN bias: [P, CK]
    _w32 = {}
    _wq = {}
    _istep = {}
    _inv = {}

    # ---------------- image loads ------------------------------------------
    x_view = td["x"].ap().rearrange("b (c p) h w -> b p c h w", p=P)
    y_view = td["y"].ap().rearrange("b (c p) h w -> b p c h w", p=P)
    xf_t = [None] * BPC
    xp_t = [None] * BPC
    h_t = [None] * BPC

    def load_x(i):
        xf = pxf.tile([P, CK, H, W], F32, name=f"xf{i}", tag="xf")
        nc.sync.dma_start(xf[:], x_view[i])
        xp = pxp.tile([P, CK, HP, WP], F16, name=f"xp{i}", tag="xp")
        nc.gpsimd.memset(xp[:], 0.0)
        nc.scalar.copy(xp[:, :, 1 : 1 + H, 1 : 1 + W], xf[:])
        xf_t[i], xp_t[i] = xf, xp

    # ---------------- per-weight quantization ------------------------------
    _step = {}

    def quant_dma(j):
        """Issue weight DMAs (sync engine only — no compute-engine stalls)."""
        w32 = wbig.tile([P, CK, C, TAPS], F32, name=f"w32_{j}", tag="wbig")
        wsrc = td[f"w{j}"].ap().rearrange("(c p) ci kh kw -> p c ci (kh kw)", p=P)
        for c in range(CK):
            for k in range(CK):
                ks = slice(k * P, (k + 1) * P)
                nc.sync.dma_start(w32[:, c, ks, :], wsrc[:, c, ks, :])
        _w32[j] = w32

    def quant_absmax(j):
        """Global absmax -> step/istep (reduces on DVE, combine on PE)."""
        w32 = _w32[j]
        pmq = []
        for c in range(CK):
            for k in range(CK):
                ks = slice(k * P, (k + 1) * P)
                ph = bnp.tile([P, 1], F32, name=f"pmq{j}_{c}_{k}", tag="pmq")
                nc.vector.tensor_reduce(
                    ph[:], w32[:, c, ks, :], axis=AX.XY, op=OP.max,
                    apply_absolute_value=True,
                )
                pmq.append(ph)
        pa = bnp.tile([P, 1], F32, name=f"pa{j}", tag="pa")
        nc.vector.tensor_max(pa[:], pmq[0][:], pmq[1][:])
        pb = bnp.tile([P, 1], F32, name=f"pb{j}", tag="pb")
        nc.vector.tensor_max(pb[:], pmq[2][:], pmq[3][:])
        pm = bnp.tile([P, 1], F32, name=f"pm{j}", tag="pm")
        nc.vector.tensor_max(pm[:], pa[:], pb[:])
        # cross-partition max via PE: transpose [128,1]->[1,128], reduce,
        # then broadcast back with a K=1 ones matmul (gpsimd ucode is ~10us)
        pmt = tpp.tile([1, P], F32, name=f"pmt{j}", tag="tp")
        nc.tensor.transpose(pmt[:], pm[:], ident32[:])
        sm = bnp.tile([1, 1], F32, name=f"sm{j}", tag="sm")
        nc.vector.tensor_reduce(sm[:], pmt[:], axis=AX.X, op=OP.max)
        pmb = tpp.tile([P, 1], F32, name=f"pmb{j}", tag="tp")
        nc.tensor.matmul(pmb[:], ones32[:], sm[:])
        pmax = bnp.tile([P, 1], F32, name=f"pmax{j}", tag="pmax")
        nc.vector.tensor_copy(pmax[:], pmb[:])
        step = const.tile([P, 1], F32, name=f"step{j}", tag=f"step{j}")
        nc.vector.tensor_scalar_mul(step[:], pmax[:], 1.0 / HALF_LVLS)
        _step[j] = step
        rmax = bnp.tile([P, 1], F32, name=f"rmax{j}", tag="rmax")
        nc.vector.reciprocal(rmax[:], pmax[:])
        istep = const.tile([P, 1], F32, name=f"istep{j}", tag=f"istep{j}")
        nc.vector.tensor_scalar_mul(istep[:], rmax[:], HALF_LVLS)
        _istep[j] = istep
        # fold step into BN scale: inv_s = inv * step
        ivs = const.tile([P, CK], F32, name=f"ivs{j}", tag=f"ivs{j}")
        nc.vector.tensor_scalar_mul(ivs[:], _inv[j][:], _step[j][:, 0:1])
        inv_s[j] = ivs

        wq = wqp.tile([P, CK, C, TAPS], F16, name=f"wq{j}", tag=f"wq{j}")
        _wq[j] = wq
        wT[j] = []
        for k in range(CK):
            wt = wtp.tile([P, CK, TAPS, P], F16, name=f"wT{j}_{k}", tag=f"wT{j}_{k}")
            wT[j].append(wt)

    def bn_prep(j):
        """BN vector prep.  Contiguous [1,256] row loads (one descriptor each
        — the [128,2] gather form is 256 tiny descriptors, ~10us), math on one
        partition, then redistribute to [128,2] via K=1 PE matmuls."""
        gv = bnp.tile([1, C], F32, name=f"gv{j}", tag=f"gv{j}")
        bev = bnp.tile([1, C], F32, name=f"bev{j}", tag=f"bev{j}")
        muv = bnp.tile([1, C], F32, name=f"muv{j}", tag=f"muv{j}")
        vav = bnp.tile([1, C], F32, name=f"vav{j}", tag=f"vav{j}")
        nc.sync.dma_start(gv[:], td[f"gamma{j}"].ap().unsqueeze(0))
        nc.sync.dma_start(bev[:], td[f"beta{j}"].ap().unsqueeze(0))
        nc.sync.dma_start(muv[:], td[f"mean{j}"].ap().unsqueeze(0))
        nc.sync.dma_start(vav[:], td[f"var{j}"].ap().unsqueeze(0))

        tv = bnp.tile([1, C], F32, name=f"tv{j}", tag="btmp")
        nc.vector.tensor_scalar_add(tv[:], vav[:], BN_EPS)
        rv = bnp.tile([1, C], F32, name=f"rv{j}", tag="btmp")
        nc.vector.reciprocal(rv[:], tv[:])
        sv = bnp.tile([1, C], F32, name=f"sv{j}", tag="btmp")
        nc.scalar.activation(sv[:], rv[:], AF.Sqrt)           # rsqrt(var+eps)
        invr = bnp.tile([1, C], F32, name=f"invr{j}", tag=f"invr{j}")
        nc.vector.tensor_mul(invr[:], sv[:], gv[:])           # gamma * rsqrt
        mi = bnp.tile([1, C], F32, name=f"mi{j}", tag="btmp")
        nc.vector.tensor_mul(mi[:], muv[:], invr[:])
        bvr = bnp.tile([1, C], F32, name=f"bvr{j}", tag=f"bvr{j}")
        nc.vector.tensor_sub(bvr[:], bev[:], mi[:])           # beta - mean*inv

        # redistribute rows -> per-partition [P, CK] with K=1 matmuls
        psB = tpp.tile([P, 2 * CK], F32, name=f"psB{j}", tag="tp")
        for c in range(CK):
            nc.tensor.matmul(
                psB[:, c : c + 1], invr[0:1, c * P : (c + 1) * P], ones32[0:1, 0:1]
            )
            nc.tensor.matmul(
                psB[:, CK + c : CK + c + 1],
                bvr[0:1, c * P : (c + 1) * P],
                ones32[0:1, 0:1],
            )
        inv = const.tile([P, CK], F32, name=f"inv{j}", tag=f"inv{j}")
        nc.vector.tensor_copy(inv[:], psB[:, 0:CK])
        bv = const.tile([P, CK], F32, name=f"bv{j}", tag=f"bv{j}")
        nc.vector.tensor_copy(bv[:], psB[:, CK : 2 * CK])
        bvec[j] = bv
        _inv[j] = inv

    def quant_chain(j):
        """Quantization pipeline at (co-chunk, ci-half) granularity so the
        first transposes (and conv matmuls) start as early as possible."""
        w32, wq, istep = _w32[j], _wq[j], _istep[j]
        for c in range(CK):      # co-chunk == psum m chunk
            for k in range(CK):  # ci half (128 input channels)
                ks = slice(k * P, (k + 1) * P)
                src = w32[:, c, ks, :]
                # wl = rne(w * istep); the +-127 clip is redundant: |w*istep|
                # <= 127*(1+2^-23) by construction, and rne of that is 127.
                # Same for the centroid's +-8 clip (|gm|/9/cstep <= 8).
                wlr = whalf.tile([P, P, TAPS], F32, name=f"wlr{j}_{c}_{k}", tag="wh")
                nc.scalar.activation(
                    wlr[:], src, AF.Identity, bias=magicv[:, 0:1], scale=istep[:, 0:1]
                )
                wl3 = whalf.tile([P, P, TAPS], F32, name=f"wl3{j}_{c}_{k}", tag="wh")
                nc.vector.tensor_scalar_sub(wl3[:], wlr[:], MAGIC)

                # per-grain (co, ci) mean over the 9 taps -> centroid levels
                gm = bnp.tile([P, P], F32, name=f"gm{j}_{c}_{k}", tag="gm")
                nc.vector.tensor_reduce(gm[:], wl3[:], axis=AX.X, op=OP.add)
                c1 = bnp.tile([P, P], F32, name=f"c1{j}_{c}_{k}", tag="c1")
                nc.vector.tensor_scalar(
                    c1[:], gm[:], 1.0 / (TAPS * CSTEP), MAGIC, OP.mult, OP.add
                )
                cent = bnp.tile([P, P], F32, name=f"cent{j}_{c}_{k}", tag="cent")
                nc.vector.tensor_scalar(
                    cent[:], c1[:], MAGIC, CSTEP, OP.subtract, OP.mult
                )
                centb = cent.unsqueeze(2).broadcast_to((P, P, TAPS))

                # dev = rne(clip(wl - cent, -63.5, 63.5)); wq = dev + cent
                dv = whalf.tile([P, P, TAPS], F32, name=f"dv{j}_{c}_{k}", tag="wh")
                nc.vector.tensor_sub(dv[:], wl3[:], centb)
                dv2 = whalf.tile([P, P, TAPS], F32, name=f"dv2{j}_{c}_{k}", tag="wh")
                nc.vector.tensor_scalar(dv2[:], dv[:], DEVW, -DEVW, OP.min, OP.max)
                dv3 = whalf.tile([P, P, TAPS], F32, name=f"dv3{j}_{c}_{k}", tag="wh")
                nc.vector.tensor_scalar(
                    dv3[:], dv2[:], MAGIC, MAGIC, OP.add, OP.subtract
                )
                nc.vector.tensor_add(wq[:, c, ks, :], dv3[:], centb)

                # PE-transpose the 9 taps of this (m=c, k): [co,ci] -> [ci,co]
                m = c
                for t0 in (0, 4, 8):
                    nb = min(4, TAPS - t0)
                    pst = tpp.tile(
                        [P, nb, P], F16, name=f"pst{j}_{m}_{k}_{t0}", tag="tp"
                    )
                    for dt in range(nb):
                        nc.tensor.transpose(
                            pst[:, dt, :],
                            wq[:, m, k * P : (k + 1) * P, t0 + dt],
                            ident16[:],
                        )
                    nc.scalar.copy(wT[j][k][:, m, t0 : t0 + nb, :], pst[:])

    # ---------------- convolutions -----------------------------------------
    def conv_mms(ps, src16, wTj, m, r0):
        idx = 0
        for k in range(CK):
            for dh in range(3):
                for dw in range(3):
                    t = dh * 3 + dw
                    nc.tensor.matmul(
                        ps[:],
                        wTj[k][:, m, t, :],
                        src16[:, k, r0 + dh : r0 + dh + NR, dw : dw + W],
                        start=(idx == 0),
                        stop=(idx == 2 * TAPS - 1),
                    )
                    idx += 1

    def conv1(i, ms=None):
        if ms is None or ms == [0]:
            hh = phh.tile([P, CK, HP, WP], F16, name=f"h{i}", tag="h")
            nc.gpsimd.memset(hh[:], 0.0)
            h_t[i] = hh
        hh = h_t[i]
        for m in (ms if ms is not None else range(CK)):
            for r in range(2):
                r0 = r * NR
                ps = psp.tile([P, NN], F32, name=f"ps1_{i}_{m}_{r}", tag="ps")
                conv_mms(ps, xp_t[i], wT[1], m, r0)
                nc.scalar.activation(
                    hh[:, m, 1 + r0 : 1 + r0 + NR, 1 : 1 + W],
                    ps.rearrange("p (r w) -> p r w", w=W),
                    AF.Relu,
                    bias=bvec[1][:, m : m + 1],
                    scale=inv_s[1][:, m : m + 1],
                )

    def conv2(i):
        for m in range(CK):
            yf = pyy.tile([P, H, W], F32, name=f"y{i}_{m}", tag="y")
            for r in range(2):
                r0 = r * NR
                ps = psp.tile([P, NN], F32, name=f"ps2_{i}_{m}_{r}", tag="ps")
                conv_mms(ps, h_t[i], wT[2], m, r0)
                t2 = pep.tile([P, NN], F32, name=f"t2_{i}_{m}_{r}", tag="t2")
                nc.scalar.activation(
                    t2[:],
                    ps[:],
                    AF.Identity,
                    bias=bvec[2][:, m : m + 1],
                    scale=inv_s[2][:, m : m + 1],
                )
                u = pep.tile([P, NN], F32, name=f"u_{i}_{m}_{r}", tag="u")
                xflat = xf_t[i][:, m, r0 : r0 + NR, :].rearrange("p r w -> p (r w)")
                nc.vector.tensor_add(u[:], t2[:], xflat)
                nc.scalar.activation(
                    yf[:, r0 : r0 + NR, :],
                    u.rearrange("p (r w) -> p r w", w=W),
                    AF.Relu,
                )
            nc.gpsimd.dma_start(y_view[i][:, m], yf[:])

    def pe_warmup(n):
        """Junk matmuls (ident16 x broadcast-ident16) to hold the PE HAM at
        K=8/8 through the head's DMA wait, so real matmuls start warm."""
        rhsb = ident16.unsqueeze(1).broadcast_to((P, 3, P))
        for i in range(n):
            scr_ps = psp.tile([P, NN], F32, name=f"warm{_wuid[0]}", tag="ps")
            _wuid[0] += 1
            nc.tensor.matmul(scr_ps[:, 0 : 3 * P], ident16[:], rhsb)

    _wuid = [0]

    # ---------------- emission order (engine priority) ---------------------
    pe_warmup(30)
    quant_dma(1)
    bn_prep(1)
    bn_prep(2)
    load_x(0)
    load_x(1)
    quant_dma(2)
    for i in range(2, BPC):
        load_x(i)
    quant_absmax(1)
    pe_warmup(40)
    quant_chain(1)
    conv1(0, ms=[0])
    conv1(1, ms=[0])
    conv1(0, ms=[1])
    conv1(1, ms=[1])
    quant_absmax(2)
    quant_chain(2)
    conv1(2)
    conv1(3)
    for i in range(BPC):
        if i + 4 < BPC:
            conv1(i + 4)
        conv2(i)


def build_bass():
    nc = bacc.Bacc(
        "TRN2", target_bir_lowering=False, debug=False, num_devices=NCORES
    )
    td = {}
    td["x"] = nc.dram_tensor("x", (BPC, C, H, W), F32, kind="ExternalInput")
    for j in (1, 2):
        td[f"w{j}"] = nc.dram_tensor(f"w{j}", (C, C, 3, 3), F32, kind="ExternalInput")
        for v in ("gamma", "beta", "mean", "var"):
            td[f"{v}{j}"] = nc.dram_tensor(f"{v}{j}", (C,), F32, kind="ExternalInput")
    td["y"] = nc.dram_tensor("y", (BPC, C, H, W), F32, kind="ExternalOutput")

    with tile.TileContext(nc) as tc:
        with ExitStack() as ctx:
            _emit(nc, tc, ctx, td)
    nc.compile()
    return nc


_NC = None


def _get_nc():
    global _NC
    if _NC is None:
        _NC = build_bass()
    return _NC


def make_in_maps(x, w1, gamma1, beta1, mean1, var1, w2, gamma2, beta2, mean2, var2):
    rep = {
        "w1": w1, "gamma1": gamma1, "beta1": beta1, "mean1": mean1, "var1": var1,
        "w2": w2, "gamma2": gamma2, "beta2": beta2, "mean2": mean2, "var2": var2,
    }
    rep = {k: np.ascontiguousarray(np.asarray(v), dtype=np.float32) for k, v in rep.items()}
    in_maps = []
    for c in range(NCORES):
        m = {"x": np.ascontiguousarray(np.asarray(x)[c * BPC : (c + 1) * BPC], dtype=np.float32)}
        m.update(rep)
        in_maps.append(m)
    return in_maps


def kernel(x, w1, gamma1, beta1, mean1, var1,
           w2, gamma2, beta2, mean2, var2, codebook=None, **_unused):
    nc = _get_nc()
    in_maps = make_in_maps(x, w1, gamma1, beta1, mean1, var1,
                           w2, gamma2, beta2, mean2, var2)
    res = run_bass_kernel_spmd(nc, in_maps, core_ids=list(range(NCORES)))
    return np.concatenate([r["y"] for r in res.results], axis=0)


# revision 17
# speedup vs baseline: 1.0982x; 1.0306x over previous
"""Trainium2 Bass kernel: quantized BasicBlock (quant-conv3x3 -> bn -> relu ->
quant-conv3x3 -> bn -> +residual -> relu).

Sharding: data-parallel over the batch dim of x across 8 NeuronCores (8 images
per core).  Weight quantization (centroid/deviation pipeline) is replicated on
every core, computed on-device.

Math notes:
  - jnp.round (round-half-even) is implemented with the fp32 magic-number
    trick: rne(v) = (v + 1.5*2^23) - 1.5*2^23 for |v| < 2^22.
  - Quantized weights are integer "levels" dev+cent = k/8 with |k| < 2048,
    exactly representable in fp16.  The global scale `step` is folded into the
    BN scale vector, so matmuls run in fp16 (4x faster than fp32 on the PE)
    with fp32 PSUM accumulation and no weight-precision loss.
"""

import sys

for _p in ("/opt/trn_rl_repo",):
    if _p not in sys.path:
        sys.path.insert(0, _p)

from contextlib import ExitStack

import numpy as np

import concourse.bass as bass
import concourse.tile as tile
from concourse import bacc, bass_isa, mybir
from concourse.bass_utils import run_bass_kernel_spmd
from concourse.masks import make_identity

P = 128
B, C, H, W = 64, 256, 28, 28
NCORES = 8
BPC = B // NCORES          # images per core
CK = C // P                # channel chunks (2)
TAPS = 9
HP, WP = H + 2, W + 2      # zero-padded spatial 30x30
NR = H // 2                # rows per psum chunk (14)
NN = NR * W                # matmul free dim (392)
F32 = mybir.dt.float32
F16 = mybir.dt.float16

MAGIC = 12582912.0         # 1.5 * 2**23  (fp32 RNE round-to-int trick)
HALF_LVLS = 127.0
LV = 8.0                   # 2**(NUM_BITS-1)
CSTEP = HALF_LVLS / LV     # 15.875
DEVW = 0.5 * HALF_LVLS     # 63.5
BN_EPS = 1e-5

AF = mybir.ActivationFunctionType
OP = mybir.AluOpType
AX = mybir.AxisListType


def _emit(nc, tc, ctx, td):
    """Emit the whole per-core program.  td: dict of DRAM tensor handles."""
    const = ctx.enter_context(tc.tile_pool(name="const", bufs=1))
    bnp = ctx.enter_context(tc.tile_pool(name="bnp", bufs=2))
    wbig = ctx.enter_context(tc.tile_pool(name="wbig", bufs=1))
    whalf = ctx.enter_context(tc.tile_pool(name="whalf", bufs=3))
    wqp = ctx.enter_context(tc.tile_pool(name="wqp", bufs=1))
    wtp = ctx.enter_context(tc.tile_pool(name="wtp", bufs=1))
    tpp = ctx.enter_context(tc.tile_pool(name="tpp", bufs=2, space="PSUM"))
    psp = ctx.enter_context(tc.tile_pool(name="psp", bufs=6, space="PSUM"))
    pxf = ctx.enter_context(tc.tile_pool(name="pxf", bufs=6))
    pxp = ctx.enter_context(tc.tile_pool(name="pxp", bufs=5))
    phh = ctx.enter_context(tc.tile_pool(name="phh", bufs=5))
    pyy = ctx.enter_context(tc.tile_pool(name="pyy", bufs=3))
    pep = ctx.enter_context(tc.tile_pool(name="pep", bufs=3))

    ident16 = const.tile([P, P], F16, name="ident16", tag="ident16")
    make_identity(nc, ident16)
    ident32 = const.tile([P, P], F32, name="ident32", tag="ident32")
    make_identity(nc, ident32)
    ones32 = const.tile([1, P], F32, name="ones32", tag="ones32")
    nc.gpsimd.memset(ones32[:], 1.0)
    magicv = const.tile([P, 1], F32, name="magicv", tag="magicv")
    nc.gpsimd.memset(magicv[:], MAGIC)
    # warm the ScalarE activation tables during the initial DMA wait so the
    # one-time ACT_TABLE_LOADs don't block the quant chain later
    scr = const.tile([P, 1], F32, name="scr", tag="scr")
    nc.scalar.activation(scr[:], magicv[:], AF.Sqrt)
    nc.scalar.activation(scr[:], magicv[:], AF.Relu)

    wT = {}      # wT[j][k] : [P(ci), CK(m), TAPS, P(co)] fp16
    inv_s = {}   # BN scale with quant step folded in: [P, CK]
    bvec = {}    # BN bias: [P, CK]
    _w32 = {}
    _wq = {}
    _istep = {}
    _inv = {}

    # ---------------- image loads ------------------------------------------
    x_view = td["x"].ap().rearrange("b (c p) h w -> b p c h w", p=P)
    y_view = td["y"].ap().rearrange("b (c p) h w -> b p c h w", p=P)
    xf_t = [None] * BPC
    xp_t = [None] * BPC
    h_t = [None] * BPC

    def load_x(i):
        xf = pxf.tile([P, CK, H, W], F32, name=f"xf{i}", tag="xf")
        nc.sync.dma_start(xf[:], x_view[i])
        xp = pxp.tile([P, CK, HP, WP], F16, name=f"xp{i}", tag="xp")
        nc.gpsimd.memset(xp[:], 0.0)
        nc.scalar.copy(xp[:, :, 1 : 1 + H, 1 : 1 + W], xf[:])
        xf_t[i], xp_t[i] = xf, xp

    # ---------------- per-weight quantization ------------------------------
    _step = {}

    def quant_dma(j):
        """Issue weight DMAs (sync engine only — no compute-engine stalls)."""
        w32 = wbig.tile([P, CK, C, TAPS], F32, name=f"w32_{j}", tag="wbig")
        wsrc = td[f"w{j}"].ap().rearrange("(c p) ci kh kw -> p c ci (kh kw)", p=P)
        for c in range(CK):
            for k in range(CK):
                ks = slice(k * P, (k + 1) * P)
                nc.sync.dma_start(w32[:, c, ks, :], wsrc[:, c, ks, :])
        _w32[j] = w32

    def quant_absmax(j):
        """Global absmax -> step/istep (reduces on DVE, combine on PE)."""
        w32 = _w32[j]
        pmq = []
        for c in range(CK):
            for k in range(CK):
                ks = slice(k * P, (k + 1) * P)
                ph = bnp.tile([P, 1], F32, name=f"pmq{j}_{c}_{k}", tag="pmq")
                nc.vector.tensor_reduce(
                    ph[:], w32[:, c, ks, :], axis=AX.XY, op=OP.max,
                    apply_absolute_value=True,
                )
                pmq.append(ph)
        pa = bnp.tile([P, 1], F32, name=f"pa{j}", tag="pa")
        nc.vector.tensor_max(pa[:], pmq[0][:], pmq[1][:])
        pb = bnp.tile([P, 1], F32, name=f"pb{j}", tag="pb")
        nc.vector.tensor_max(pb[:], pmq[2][:], pmq[3][:])
        pm = bnp.tile([P, 1], F32, name=f"pm{j}", tag="pm")
        nc.vector.tensor_max(pm[:], pa[:], pb[:])
        # cross-partition max via PE: transpose [128,1]->[1,128], reduce,
        # then broadcast back with a K=1 ones matmul (gpsimd ucode is ~10us)
        pmt = tpp.tile([1, P], F32, name=f"pmt{j}", tag="tp")
        nc.tensor.transpose(pmt[:], pm[:], ident32[:])
        sm = bnp.tile([1, 1], F32, name=f"sm{j}", tag="sm")
        nc.vector.tensor_reduce(sm[:], pmt[:], axis=AX.X, op=OP.max)
        pmb = tpp.tile([P, 1], F32, name=f"pmb{j}", tag="tp")
        nc.tensor.matmul(pmb[:], ones32[:], sm[:])
        pmax = bnp.tile([P, 1], F32, name=f"pmax{j}", tag="pmax")
        nc.vector.tensor_copy(pmax[:], pmb[:])
        step = const.tile([P, 1], F32, name=f"step{j}", tag=f"step{j}")
        nc.vector.tensor_scalar_mul(step[:], pmax[:], 1.0 / HALF_LVLS)
        _step[j] = step
        rmax = bnp.tile([P, 1], F32, name=f"rmax{j}", tag="rmax")
        nc.vector.reciprocal(rmax[:], pmax[:])
        istep = const.tile([P, 1], F32, name=f"istep{j}", tag=f"istep{j}")
        nc.vector.tensor_scalar_mul(istep[:], rmax[:], HALF_LVLS)
        _istep[j] = istep
        if j == 1:
            i16 = const.tile([P, 1], F16, name="istep16", tag="istep16")
            nc.vector.tensor_copy(i16[:], istep[:])
            _istep16[0] = i16
        # fold step into BN scale: inv_s = inv * step
        ivs = const.tile([P, CK], F32, name=f"ivs{j}", tag=f"ivs{j}")
        nc.vector.tensor_scalar_mul(ivs[:], _inv[j][:], _step[j][:, 0:1])
        inv_s[j] = ivs

        wq = wqp.tile([P, CK, C, TAPS], F16, name=f"wq{j}", tag=f"wq{j}")
        _wq[j] = wq
        wT[j] = []
        for k in range(CK):
            wt = wtp.tile([P, CK, TAPS, P], F16, name=f"wT{j}_{k}", tag=f"wT{j}_{k}")
            wT[j].append(wt)

    def bn_prep(j):
        """BN vector prep.  Contiguous [1,256] row loads (one descriptor each
        — the [128,2] gather form is 256 tiny descriptors, ~10us), math on one
        partition, then redistribute to [128,2] via K=1 PE matmuls."""
        gv = bnp.tile([1, C], F32, name=f"gv{j}", tag=f"gv{j}")
        bev = bnp.tile([1, C], F32, name=f"bev{j}", tag=f"bev{j}")
        muv = bnp.tile([1, C], F32, name=f"muv{j}", tag=f"muv{j}")
        vav = bnp.tile([1, C], F32, name=f"vav{j}", tag=f"vav{j}")
        nc.sync.dma_start(gv[:], td[f"gamma{j}"].ap().unsqueeze(0))
        nc.sync.dma_start(bev[:], td[f"beta{j}"].ap().unsqueeze(0))
        nc.sync.dma_start(muv[:], td[f"mean{j}"].ap().unsqueeze(0))
        nc.sync.dma_start(vav[:], td[f"var{j}"].ap().unsqueeze(0))

        # redistribute the raw rows -> [P, 4, CK] via K=1 PE matmuls FIRST;
        # single-partition DVE ops are ~20x slower than full-width ones, so
        # all the math happens after the spread.
        psB = tpp.tile([P, 4 * CK], F32, name=f"psB{j}", tag="tp")
        for v, row in enumerate((gv, bev, muv, vav)):
            for c in range(CK):
                nc.tensor.matmul(
                    psB[:, v * CK + c : v * CK + c + 1],
                    row[0:1, c * P : (c + 1) * P],
                    ones32[0:1, 0:1],
                )
        bn4 = bnp.tile([P, 4, CK], F32, name=f"bn4_{j}", tag=f"bn4_{j}")
        nc.vector.tensor_copy(bn4[:], psB[:].rearrange("p (v c) -> p v c", c=CK))
        gvp, bevp, muvp, vavp = (bn4[:, v, :] for v in range(4))

        tv = bnp.tile([P, CK], F32, name=f"tv{j}", tag="btmp")
        nc.vector.tensor_scalar_add(tv[:], vavp, BN_EPS)
        rv = bnp.tile([P, CK], F32, name=f"rv{j}", tag="btmp")
        nc.vector.reciprocal(rv[:], tv[:])
        sv = bnp.tile([P, CK], F32, name=f"sv{j}", tag="btmp")
        nc.scalar.activation(sv[:], rv[:], AF.Sqrt)           # rsqrt(var+eps)
        inv = const.tile([P, CK], F32, name=f"inv{j}", tag=f"inv{j}")
        nc.vector.tensor_mul(inv[:], sv[:], gvp)              # gamma * rsqrt
        mi = bnp.tile([P, CK], F32, name=f"mi{j}", tag="btmp")
        nc.vector.tensor_mul(mi[:], muvp, inv[:])
        bv = const.tile([P, CK], F32, name=f"bv{j}", tag=f"bv{j}")
        nc.vector.tensor_sub(bv[:], bevp, mi[:])              # beta - mean*inv
        bvec[j] = bv
        _inv[j] = inv

    def quant_chain(j):
        """Quantization pipeline at (co-chunk, ci-half) granularity so the
        first transposes (and conv matmuls) start as early as possible."""
        w32, wq, istep = _w32[j], _wq[j], _istep[j]
        for c in range(CK):      # co-chunk == psum m chunk
            for k in range(CK):  # ci half (128 input channels)
                ks = slice(k * P, (k + 1) * P)
                src = w32[:, c, ks, :]
                # wl = rne(w * istep); the +-127 clip is redundant: |w*istep|
                # <= 127*(1+2^-23) by construction, and rne of that is 127.
                # Same for the centroid's +-8 clip (|gm|/9/cstep <= 8).
                wlr = whalf.tile([P, P, TAPS], F32, name=f"wlr{j}_{c}_{k}", tag="wh")
                nc.scalar.activation(
                    wlr[:], src, AF.Identity, bias=magicv[:, 0:1], scale=istep[:, 0:1]
                )
                wl3 = whalf.tile([P, P, TAPS], F32, name=f"wl3{j}_{c}_{k}", tag="wh")
                nc.vector.tensor_scalar_sub(wl3[:], wlr[:], MAGIC)

                # per-grain (co, ci) mean over the 9 taps -> centroid levels
                gm = bnp.tile([P, P], F32, name=f"gm{j}_{c}_{k}", tag="gm")
                nc.vector.tensor_reduce(gm[:], wl3[:], axis=AX.X, op=OP.add)
                c1 = bnp.tile([P, P], F32, name=f"c1{j}_{c}_{k}", tag="c1")
                nc.vector.tensor_scalar(
                    c1[:], gm[:], 1.0 / (TAPS * CSTEP), MAGIC, OP.mult, OP.add
                )
                cent = bnp.tile([P, P], F32, name=f"cent{j}_{c}_{k}", tag="cent")
                nc.vector.tensor_scalar(
                    cent[:], c1[:], MAGIC, CSTEP, OP.subtract, OP.mult
                )
                centb = cent.unsqueeze(2).broadcast_to((P, P, TAPS))

                # dev = rne(clip(wl - cent, -63.5, 63.5)); wq = dev + cent
                dv = whalf.tile([P, P, TAPS], F32, name=f"dv{j}_{c}_{k}", tag="wh")
                nc.vector.tensor_sub(dv[:], wl3[:], centb)
                dv2 = whalf.tile([P, P, TAPS], F32, name=f"dv2{j}_{c}_{k}", tag="wh")
                nc.vector.tensor_scalar(dv2[:], dv[:], DEVW, -DEVW, OP.min, OP.max)
                dv3 = whalf.tile([P, P, TAPS], F32, name=f"dv3{j}_{c}_{k}", tag="wh")
                nc.vector.tensor_scalar(
                    dv3[:], dv2[:], MAGIC, MAGIC, OP.add, OP.subtract
                )
                nc.vector.tensor_add(wq[:, c, ks, :], dv3[:], centb)

                # PE-transpose the 9 taps of this (m=c, k): [co,ci] -> [ci,co]
                m = c
                for t0 in (0, 4, 8):
                    nb = min(4, TAPS - t0)
                    pst = tpp.tile(
                        [P, nb, P], F16, name=f"pst{j}_{m}_{k}_{t0}", tag="tp"
                    )
                    for dt in range(nb):
                        nc.tensor.transpose(
                            pst[:, dt, :],
                            wq[:, m, k * P : (k + 1) * P, t0 + dt],
                            ident16[:],
                        )
                    nc.scalar.copy(wT[j][k][:, m, t0 : t0 + nb, :], pst[:])

    # ---------------- convolutions -----------------------------------------
    def conv_mms(ps, src16, wTj, m, r0):
        idx = 0
        for k in range(CK):
            for dh in range(3):
                for dw in range(3):
                    t = dh * 3 + dw
                    nc.tensor.matmul(
                        ps[:],
                        wTj[k][:, m, t, :],
                        src16[:, k, r0 + dh : r0 + dh + NR, dw : dw + W],
                        start=(idx == 0),
                        stop=(idx == 2 * TAPS - 1),
                    )
                    idx += 1

    def conv1(i, ms=None):
        if ms is None or ms == [0]:
            hh = phh.tile([P, CK, HP, WP], F16, name=f"h{i}", tag="h")
            nc.gpsimd.memset(hh[:], 0.0)
            h_t[i] = hh
        hh = h_t[i]
        for m in (ms if ms is not None else range(CK)):
            for r in range(2):
                r0 = r * NR
                ps = psp.tile([P, NN], F32, name=f"ps1_{i}_{m}_{r}", tag="ps")
                conv_mms(ps, xp_t[i], wT[1], m, r0)
                nc.scalar.activation(
                    hh[:, m, 1 + r0 : 1 + r0 + NR, 1 : 1 + W],
                    ps.rearrange("p (r w) -> p r w", w=W),
                    AF.Relu,
                    bias=bvec[1][:, m : m + 1],
                    scale=inv_s[1][:, m : m + 1],
                )

    def conv2(i):
        for m in range(CK):
            yf = pyy.tile([P, H, W], F32, name=f"y{i}_{m}", tag="y")
            for r in range(2):
                r0 = r * NR
                ps = psp.tile([P, NN], F32, name=f"ps2_{i}_{m}_{r}", tag="ps")
                conv_mms(ps, h_t[i], wT[2], m, r0)
                t2 = pep.tile([P, NN], F32, name=f"t2_{i}_{m}_{r}", tag="t2")
                nc.scalar.activation(
                    t2[:],
                    ps[:],
                    AF.Identity,
                    bias=bvec[2][:, m : m + 1],
                    scale=inv_s[2][:, m : m + 1],
                )
                u = pep.tile([P, NN], F32, name=f"u_{i}_{m}_{r}", tag="u")
                xflat = xf_t[i][:, m, r0 : r0 + NR, :].rearrange("p r w -> p (r w)")
                nc.vector.tensor_add(u[:], t2[:], xflat)
                nc.scalar.activation(
                    yf[:, r0 : r0 + NR, :],
                    u.rearrange("p (r w) -> p r w", w=W),
                    AF.Relu,
                )
            nc.gpsimd.dma_start(y_view[i][:, m], yf[:])

    def pe_warmup(n, gated=False):
        """Junk matmuls to hold the PE HAM at K=8/8 through the head's DMA
        wait, so real matmuls start warm.  gated=True makes them depend on
        istep so the scheduler cannot run them before the absmax path."""
        for i in range(n):
            scr_ps = psp.tile([P, NN], F32, name=f"warm{_wuid[0]}", tag="ps")
            _wuid[0] += 1
            if gated:
                rhsb = _istep16[0].broadcast_to((P, 3 * P))
            else:
                rhsb = ident16.unsqueeze(1).broadcast_to((P, 3, P))
            nc.tensor.matmul(scr_ps[:, 0 : 3 * P], ident16[:], rhsb)

    _wuid = [0]
    _istep16 = [None]

    # ---------------- emission order (engine priority) ---------------------
    pe_warmup(30)
    quant_dma(1)
    bn_prep(1)
    bn_prep(2)
    load_x(0)
    load_x(1)
    quant_dma(2)
    for i in range(2, BPC):
        load_x(i)
    quant_absmax(1)
    pe_warmup(45, gated=True)
    quant_chain(1)
    conv1(0, ms=[0])
    conv1(1, ms=[0])
    conv1(0, ms=[1])
    conv1(1, ms=[1])
    quant_absmax(2)
    quant_chain(2)
    conv1(2)
    conv1(3)
    for i in range(BPC):
        if i + 4 < BPC:
            conv1(i + 4)
        conv2(i)


def build_bass():
    nc = bacc.Bacc(
        "TRN2", target_bir_lowering=False, debug=False, num_devices=NCORES
    )
    td = {}
    td["x"] = nc.dram_tensor("x", (BPC, C, H, W), F32, kind="ExternalInput")
    for j in (1, 2):
        td[f"w{j}"] = nc.dram_tensor(f"w{j}", (C, C, 3, 3), F32, kind="ExternalInput")
        for v in ("gamma", "beta", "mean", "var"):
            td[f"{v}{j}"] = nc.dram_tensor(f"{v}{j}", (C,), F32, kind="ExternalInput")
    td["y"] = nc.dram_tensor("y", (BPC, C, H, W), F32, kind="ExternalOutput")

    with tile.TileContext(nc) as tc:
        with ExitStack() as ctx:
            _emit(nc, tc, ctx, td)
    nc.compile()
    return nc


_NC = None


def _get_nc():
    global _NC
    if _NC is None:
        _NC = build_bass()
    return _NC


def make_in_maps(x, w1, gamma1, beta1, mean1, var1, w2, gamma2, beta2, mean2, var2):
    rep = {
        "w1": w1, "gamma1": gamma1, "beta1": beta1, "mean1": mean1, "var1": var1,
        "w2": w2, "gamma2": gamma2, "beta2": beta2, "mean2": mean2, "var2": var2,
    }
    rep = {k: np.ascontiguousarray(np.asarray(v), dtype=np.float32) for k, v in rep.items()}
    in_maps = []
    for c in range(NCORES):
        m = {"x": np.ascontiguousarray(np.asarray(x)[c * BPC : (c + 1) * BPC], dtype=np.float32)}
        m.update(rep)
        in_maps.append(m)
    return in_maps


def kernel(x, w1, gamma1, beta1, mean1, var1,
           w2, gamma2, beta2, mean2, var2, codebook=None, **_unused):
    nc = _get_nc()
    in_maps = make_in_maps(x, w1, gamma1, beta1, mean1, var1,
                           w2, gamma2, beta2, mean2, var2)
    res = run_bass_kernel_spmd(nc, in_maps, core_ids=list(range(NCORES)))
    return np.concatenate([r["y"] for r in res.results], axis=0)
